# revision 6
# baseline (speedup 1.0000x reference)
"""Trainium2 Bass kernel for ConstraintPredictorGNN (2x NNConv + candidate MLP).

Strategy: shard destination nodes (2500/core) + their incoming edges across 8
cores; edges bin-packed into 21 windows of <=128 dst nodes / <=1024 edges,
aggregated via one-hot matmuls in PSUM; root term via per-window pseudo-edge
tiles. AllGather of node features between convs. Candidate pairs sharded
8x65536 (padded), pairwise features gathered with dma_gather from a packed
[20480, 64] u-table. Edge-MLP matmuls run as float32r (full PE rate).
"""
import sys
sys.path.insert(0, '/opt/trn_rl_repo')
import numpy as np
from contextlib import ExitStack

import concourse.bass as bass
import concourse.tile as tile
from concourse import bacc, mybir
from concourse.library_config import mlp as mlp_lib

F32 = mybir.dt.float32
F32R = mybir.dt.float32r
I16 = mybir.dt.int16
I32 = mybir.dt.int32

N, E, P = 20000, 150000, 500000
IN_CH = HID = 32
EDGE_DIM = 16
NCLS = 4
R = 8                      # cores
OWN = N // R               # 2500 owned nodes / core
OWNP = 2560                # padded own rows (AllGather block)
NODEROWS = OWNP * R        # 20480 rows in gathered tables
NW = 21                    # windows per core
WCAP = 1024                # edge capacity per window (8 tiles)
WT = 9                     # tiles per window (8 edge + 1 pseudo)
TILES = NW * WT            # 189
EPAD = NW * WCAP           # 21504 edge slots
SLOTS = TILES * 128        # 24192 gather rows for conv2
PPAD = 65536               # padded candidate pairs per core (8 x 8192)
GCH = 8192                 # dma_gather chunk
OOB = 1 << 20


def _frow(n):
    """node id -> row in the rank-concatenated [NODEROWS, 64] tables."""
    return (n // OWN) * OWNP + (n % OWN)


def _pack_windows(deg):
    """FFD bin-pack own-local nodes into NW windows (<=128 nodes, <=WCAP edges).
    Returns list of node-id lists (own-local ids)."""
    order = np.argsort(-deg, kind="stable")
    be = np.zeros(NW, np.int64)
    bn = np.zeros(NW, np.int64)
    wnodes = [[] for _ in range(NW)]
    for nl in order:
        d = int(deg[nl])
        # best-fit: among bins with room, pick fewest nodes (ties: most edge room)
        cand = [w for w in range(NW) if be[w] + d <= WCAP and bn[w] < 128]
        if not cand:
            raise AssertionError("window packing failed")
        w = min(cand, key=lambda w: (bn[w], be[w]))
        be[w] += d
        bn[w] += 1
        wnodes[w].append(int(nl))
    return wnodes


def _prep_core(r, x, edge_index, edge_attr, candidates):
    src, dst = edge_index[0], edge_index[1]
    lo, hi = r * OWN, (r + 1) * OWN
    mask = (dst >= lo) & (dst < hi)
    esel = np.nonzero(mask)[0]
    dloc = dst[esel] - lo
    deg = np.bincount(dloc, minlength=OWN)
    wnodes = _pack_windows(deg)

    # per-node edge lists (own-local)
    order = np.argsort(dloc, kind="stable")
    esorted = esel[order]
    starts = np.searchsorted(dloc[order], np.arange(OWN))
    ends = np.searchsorted(dloc[order], np.arange(OWN) + 1)

    eaT = np.zeros((EDGE_DIM, EPAD), np.float32)
    xs1 = np.zeros((128, TILES * IN_CH), np.float32)
    dstloc = np.full((128, TILES), -1.0, np.float32)
    gidx_rows = np.zeros(SLOTS, np.int64)          # table rows for conv2 gather
    slotnode = np.full((128, NW), OOB, np.int32)   # scatter target (own-local)

    for w, nodes in enumerate(wnodes):
        eids = np.concatenate([esorted[starts[nl]:ends[nl]] for nl in nodes]) \
            if nodes else np.zeros(0, np.int64)
        slot_of = {nl: j for j, nl in enumerate(nodes)}
        ne = len(eids)
        assert ne <= WCAP
        ebase = w * WCAP
        eaT[:, ebase:ebase + ne] = edge_attr[eids].T
        dl = np.array([slot_of[int(d) - lo] for d in dst[eids]], np.float64)
        es = src[eids]
        for s in range(8):
            tl = w * WT + s
            seg = slice(s * 128, min((s + 1) * 128, ne))
            k = max(0, seg.stop - seg.start)
            if k > 0:
                dstloc[:k, tl] = dl[seg]
                xs1[:k, tl * IN_CH:(tl + 1) * IN_CH] = x[es[seg]]
                gidx_rows[tl * 128:tl * 128 + k] = _frow(es[seg] + 0)
        # pseudo tile
        tl = w * WT + 8
        nn = len(nodes)
        gl = np.array(nodes, np.int64) + lo
        dstloc[:nn, tl] = np.arange(nn)
        xs1[:nn, tl * IN_CH:(tl + 1) * IN_CH] = x[gl]
        gidx_rows[tl * 128:tl * 128 + nn] = _frow(gl)
        slotnode[:nn, w] = np.array(nodes, np.int32)

    # wrap gather indices: one dma_gather call per window (1152 idx each)
    def wrap(ix):
        n = len(ix)
        return np.tile(ix.astype(np.int16).reshape(n // 16, 16).T, (8, 1))
    g2 = np.concatenate(
        [wrap(gidx_rows[w * WT * 128:(w + 1) * WT * 128]) for w in range(NW)],
        axis=1)  # [128, 21*72]

    # candidates shard
    c = candidates[r * (P // R):(r + 1) * (P // R)]
    c0 = np.zeros(PPAD, np.int64)
    c1 = np.zeros(PPAD, np.int64)
    c0[:len(c)] = _frow(c[:, 0].astype(np.int64))
    c1[:len(c)] = _frow(c[:, 1].astype(np.int64))
    gc = np.concatenate(
        [wrap(c0[i * GCH:(i + 1) * GCH]) for i in range(PPAD // GCH)] +
        [wrap(c1[i * GCH:(i + 1) * GCH]) for i in range(PPAD // GCH)], axis=1)

    return dict(eaT=eaT, xs1=xs1, dstloc=dstloc, gidx2=g2, slotnode=slotnode,
                candidx=gc)


def _prep_shared(ws):
    def rep(v, n=128):
        return np.tile(np.asarray(v, np.float32).reshape(1, -1), (n, 1))
    out = {}
    for ci, pre in ((1, "c1"), (2, "c2")):
        w1 = ws[f"{pre}_w1"]; b1 = ws[f"{pre}_b1"]
        w2 = ws[f"{pre}_w2"]; b2 = ws[f"{pre}_b2"]
        root = ws[f"{pre}_root"]; bias = ws[f"{pre}_bias"]
        out[f"w1_{ci}"] = np.ascontiguousarray(w1)                       # [16,1024]
        out[f"w2_{ci}"] = np.ascontiguousarray(
            w2.reshape(8, 128, 1024).transpose(1, 0, 2).reshape(128, 8192))
        out[f"b1c_{ci}"] = np.ascontiguousarray(b1.reshape(8, 128).T)    # [128,8]
        out[f"b2rep_{ci}"] = rep(b2)                                     # [128,1024]
        out[f"rootrep_{ci}"] = rep(root.reshape(-1))                     # [128,1024]
        out[f"biasrep_{ci}"] = rep(bias)                                 # [128,32]
    mw1 = np.concatenate([ws["m_w1"][:HID, :], ws["m_w1"][HID:, :]], axis=1)
    out["mw1"] = np.ascontiguousarray(mw1)                               # [32,64]
    out["b1rep64"] = rep(np.concatenate([ws["m_b1"], np.zeros(HID, np.float32)]))
    out["w2rep"] = rep(ws["m_w2"].T.reshape(-1))                         # [128,128]
    out["b2rep4"] = rep(ws["m_b2"])                                      # [128,4]
    out["iota"] = np.tile(np.arange(128, dtype=np.float32)[None, :], (128, 1))
    out["ident"] = np.eye(128, dtype=np.float32)
    return out


# ----------------------------------------------------------------- device code

def _conv(nc, tc, ctx, cidx, *, w1, w2r, b1c, b2rep, rootrep, biasrep,
          ea_dram, dstloc, iota_f, get_xs, h_slots, slotnode_sb, h_dense):
    """One NNConv layer: writes relu(agg + root + bias) rows into h_dense.
    get_xs(w) -> accessor(s) giving the [128, 32] source-feature AP of tile s."""
    pools = ExitStack()
    h1p = pools.enter_context(tc.tile_pool(name=f"h1p{cidx}", bufs=1))
    eap = pools.enter_context(tc.tile_pool(name=f"eap{cidx}", bufs=2))
    wp = pools.enter_context(tc.tile_pool(name=f"wp{cidx}", bufs=3))
    mp = pools.enter_context(tc.tile_pool(name=f"mp{cidx}", bufs=3))
    ps_h1 = pools.enter_context(tc.tile_pool(name=f"psh1_{cidx}", bufs=2, space="PSUM"))
    ps_w = pools.enter_context(tc.tile_pool(name=f"psw{cidx}", bufs=2, space="PSUM"))
    ps_a = pools.enter_context(tc.tile_pool(name=f"psa{cidx}", bufs=2, space="PSUM"))

    nc.vector.memset(h_slots[:], 0.0)
    for w in range(NW):
        agg = ps_a.tile([128, HID], F32, tag="agg")
        ea_w = eap.tile([EDGE_DIM, WCAP], F32R, tag="ea_w", name="ea_w")
        nc.sync.dma_start(ea_w[:], ea_dram[:, w * WCAP:(w + 1) * WCAP])
        xs_of = get_xs(w)
        h1t = [None, None]
        for st in range(2):
            # layer1: h1T chunks for 512 edges
            h1t[st] = h1p.tile([128, 8, 512], F32R, tag=f"h1t{st}", name=f"h1t{st}")
            for kc in range(8):
                p1 = ps_h1.tile([128, 512], F32, tag="p1")
                nc.tensor.matmul(
                    p1[:], w1[:, kc * 128:(kc + 1) * 128],
                    ea_w[:, st * 512:(st + 1) * 512],
                    start=True, stop=True)
                nc.scalar.activation(h1t[st][:, kc, :], p1[:],
                                     mybir.ActivationFunctionType.Relu,
                                     bias=b1c[:, kc:kc + 1])
        for s in range(WT):
            t = w * WT + s
            if s < 8:
                st, q = s // 4, s % 4
                # layer2: W = h1 @ w2 + b2 for this 128-edge tile
                pw = ps_w.tile([128, 1024], F32, tag="pw")
                for kc in range(8):
                    lhsT = h1t[st][:, kc, q * 128:(q + 1) * 128]
                    for hf in range(2):
                        nc.tensor.matmul(
                            pw[:, hf * 512:(hf + 1) * 512], lhsT,
                            w2r[:, kc * 1024 + hf * 512: kc * 1024 + (hf + 1) * 512],
                            start=(kc == 0), stop=(kc == 7))
                wt = wp.tile([128, 1024], F32, tag="wt")
                nc.vector.tensor_add(wt[:], pw[:], b2rep[:])
                wsrc = wt[:]
            else:
                wsrc = rootrep[:]
            xs = xs_of(s)
            tmp = mp.tile([128, HID, HID], F32, tag="tmp")
            nc.vector.tensor_tensor(
                out=tmp[:], in0=wsrc.rearrange("p (i o) -> p o i", o=HID),
                in1=xs.unsqueeze(1).broadcast_to([128, HID, HID]),
                op=mybir.AluOpType.mult)
            msg = mp.tile([128, HID], F32, tag="msg")
            nc.vector.tensor_reduce(msg[:], tmp[:], axis=mybir.AxisListType.X,
                                    op=mybir.AluOpType.add)
            oh = mp.tile([128, 128], F32, tag="oh")
            nc.vector.tensor_scalar(oh[:], iota_f[:], dstloc[:, t:t + 1], None,
                                    op0=mybir.AluOpType.is_equal)
            nc.tensor.matmul(agg[:], oh[:], msg[:], start=(s == 0), stop=(s == 8))
        hb = mp.tile([128, HID], F32, tag="hb")
        nc.vector.tensor_add(hb[:], agg[:], biasrep[:])
        nc.vector.tensor_scalar_max(h_slots[:, w * 64: w * 64 + HID], hb[:], 0.0)
        nc.gpsimd.indirect_dma_start(
            out=h_dense, in_=h_slots[:, w * 64:(w + 1) * 64],
            out_offset=bass.IndirectOffsetOnAxis(ap=slotnode_sb[:, w:w + 1], axis=0),
            in_offset=None, bounds_check=OWNP - 1, oob_is_err=False)
    pools.close()


def build_nc():
    nc = bacc.Bacc("TRN2", target_bir_lowering=False, debug=False, num_devices=R)
    di = {}

    def inp(name, shape, dt):
        di[name] = nc.dram_tensor(name, shape, dt, kind="ExternalInput").ap()
        return di[name]

    # per-core data
    inp("eaT", [EDGE_DIM, EPAD], F32R)
    inp("xs1", [128, TILES * IN_CH], F32)
    inp("dstloc", [128, TILES], F32)
    inp("gidx2", [128, SLOTS // 16], I16)
    inp("slotnode", [128, NW], I32)
    inp("candidx", [128, 2 * PPAD // 16], I16)
    # shared weights/consts
    for ci in (1, 2):
        inp(f"w1_{ci}", [EDGE_DIM, 1024], F32R)
        inp(f"w2_{ci}", [128, 8192], F32R)
        inp(f"b1c_{ci}", [128, 8], F32)
        inp(f"b2rep_{ci}", [128, 1024], F32)
        inp(f"rootrep_{ci}", [128, 1024], F32)
        inp(f"biasrep_{ci}", [128, HID], F32)
    inp("mw1", [HID, 2 * HID], F32R)
    inp("b1rep64", [128, 2 * HID], F32)
    inp("w2rep", [128, NCLS * HID], F32)
    inp("b2rep4", [128, NCLS], F32)
    inp("iota", [128, 128], F32)
    inp("ident", [128, 128], F32)

    h_out = nc.dram_tensor("h_out", [NODEROWS, 64], F32, kind="ExternalOutput").ap()
    lg_out = nc.dram_tensor("lg_out", [PPAD, NCLS], F32, kind="ExternalOutput").ap()

    h_dense1 = nc.dram_tensor("h_dense1", [OWNP, 64], F32).ap()
    h_dense2 = nc.dram_tensor("h_dense2", [OWNP, 64], F32).ap()
    h_full1 = nc.dram_tensor("h_full1", [NODEROWS, 64], F32, addr_space="Shared").ap()
    h_full2 = nc.dram_tensor("h_full2", [NODEROWS, 64], F32, addr_space="Shared").ap()
    u_full = nc.dram_tensor("u_full", [NODEROWS, 2 * HID], F32).ap()

    with tile.TileContext(nc) as tc, ExitStack() as top:
        cpool = top.enter_context(tc.tile_pool(name="consts", bufs=1))
        with tc.tile_critical():
            nc.gpsimd.load_library(mlp_lib)

        def load(name, dt=None, shape=None):
            t = cpool.tile(shape or [s for s in di[name].shape],
                           dt or di[name].dtype, tag=name, name=name)
            nc.sync.dma_start(t[:], di[name])
            return t

        iota_f = load("iota")
        ident = load("ident")
        slotnode_sb = load("slotnode")
        dstloc = load("dstloc")
        h_slots = cpool.tile([128, NW * 64], F32, tag="h_slots")

        # ---------------- conv1 ----------------
        with ExitStack() as c1s:
            bp = c1s.enter_context(tc.tile_pool(name="big1", bufs=1))
            def get_xs1(w):
                xw = bp.tile([128, WT * IN_CH], F32, tag="xs1w", name="xs1w", bufs=3)
                nc.sync.dma_start(
                    xw[:], di["xs1"][:, w * WT * IN_CH:(w + 1) * WT * IN_CH])
                return lambda s: xw[:, s * IN_CH:(s + 1) * IN_CH]
            w1 = load("w1_1")
            w2r = bp.tile([128, 8192], F32R, tag="w2r1")
            nc.sync.dma_start(w2r[:], di["w2_1"])
            b1c = load("b1c_1"); b2rep = load("b2rep_1")
            rootrep = load("rootrep_1"); biasrep = load("biasrep_1")
            _conv(nc, tc, c1s, 1, w1=w1[:], w2r=w2r[:], b1c=b1c[:], b2rep=b2rep,
                  rootrep=rootrep, biasrep=biasrep, ea_dram=di["eaT"],
                  dstloc=dstloc[:], iota_f=iota_f, get_xs=get_xs1,
                  h_slots=h_slots[:], slotnode_sb=slotnode_sb[:], h_dense=h_dense1)
            nc.gpsimd.collective_compute(
                "AllGather", mybir.AluOpType.bypass,
                replica_groups=[list(range(R))], ins=[h_dense1], outs=[h_full1])

            # ---------------- conv2 ----------------
            gidx2 = bp.tile([128, SLOTS // 16], I16, tag="gidx2", name="gidx2")
            nc.sync.dma_start(gidx2[:], di["gidx2"])
            NWI = WT * 128 // 16   # idx cols per window (72)
            def get_xs2(w):
                xw = bp.tile([128, WT, 64], F32, tag="xs2w", name="xs2w", bufs=3)
                nc.gpsimd.dma_gather(
                    out_ap=xw[:], in_ap=h_full1,
                    idxs_ap=gidx2[:, w * NWI:(w + 1) * NWI],
                    num_idxs=WT * 128, num_idxs_reg=WT * 128, elem_size=64,
                    single_packet=False)
                return lambda s: xw[:, s, 0:HID]
            w1b = load("w1_2")
            w2rb = bp.tile([128, 8192], F32R, tag="w2r1")  # reuse slot
            nc.sync.dma_start(w2rb[:], di["w2_2"])
            b1cb = load("b1c_2"); b2repb = load("b2rep_2")
            rootrepb = load("rootrep_2"); biasrepb = load("biasrep_2")
            _conv(nc, tc, c1s, 2, w1=w1b[:], w2r=w2rb[:], b1c=b1cb[:], b2rep=b2repb,
                  rootrep=rootrepb, biasrep=biasrepb, ea_dram=di["eaT"],
                  dstloc=dstloc[:], iota_f=iota_f, get_xs=get_xs2,
                  h_slots=h_slots[:], slotnode_sb=slotnode_sb[:], h_dense=h_dense2)
            nc.gpsimd.collective_compute(
                "AllGather", mybir.AluOpType.bypass,
                replica_groups=[list(range(R))], ins=[h_dense2], outs=[h_full2])
            nc.sync.dma_start(h_out, h_full2)

        # ---------------- u = [h @ mw1_top + b1, h @ mw1_bot] ----------------
        with ExitStack() as us:
            up = us.enter_context(tc.tile_pool(name="upool", bufs=3))
            ups = us.enter_context(tc.tile_pool(name="upsum", bufs=2, space="PSUM"))
            mw1 = load("mw1")
            b1rep64 = load("b1rep64")
            for ti in range(NODEROWS // 128):
                ht = up.tile([128, 64], F32, tag="ht")
                nc.sync.dma_start(ht[:], h_full2[ti * 128:(ti + 1) * 128, :])
                pt = ups.tile([32, 128], F32, tag="pt")
                nc.tensor.transpose(pt[:], ht[:, 0:HID], ident[:])
                hT = up.tile([32, 128], F32R, tag="hT")
                nc.vector.tensor_copy(hT[:], pt[:])
                pu = ups.tile([128, 64], F32, tag="pu")
                nc.tensor.matmul(pu[:], hT[:], mw1[:], start=True, stop=True)
                ut = up.tile([128, 64], F32, tag="ut")
                nc.vector.tensor_add(ut[:], pu[:], b1rep64[:])
                nc.sync.dma_start(u_full[ti * 128:(ti + 1) * 128, :], ut[:])

        # ---------------- candidate MLP ----------------
        with ExitStack() as cs:
            gp = cs.enter_context(tc.tile_pool(name="gp", bufs=2))
            zp = cs.enter_context(tc.tile_pool(name="zp", bufs=4))
            cip = cs.enter_context(tc.tile_pool(name="cip", bufs=1))
            candidx = cip.tile([128, 2 * PPAD // 16], I16, tag="candidx", name="candidx")
            nc.sync.dma_start(candidx[:], di["candidx"])
            w2rep = load("w2rep")
            b2rep4 = load("b2rep4")
            NCHUNK = PPAD // GCH
            for ch in range(NCHUNK):
                g0 = gp.tile([128, GCH // 128, 64], F32, tag="g0")
                g1 = gp.tile([128, GCH // 128, 64], F32, tag="g1")
                for g, base in ((g0, 0), (g1, PPAD // 16)):
                    nc.gpsimd.dma_gather(
                        out_ap=g[:], in_ap=u_full,
                        idxs_ap=candidx[:, base + ch * (GCH // 16):
                                        base + (ch + 1) * (GCH // 16)],
                        num_idxs=GCH, num_idxs_reg=GCH, elem_size=64,
                        single_packet=False)
                lgc = zp.tile([128, (GCH // 128) * NCLS], F32, tag="lgc")
                for j in range(GCH // 128):
                    za = zp.tile([128, HID], F32, tag="za")
                    nc.vector.tensor_add(za[:], g0[:, j, 0:HID], g1[:, j, HID:64])
                    z = zp.tile([128, HID], F32, tag="z")
                    nc.vector.tensor_scalar_max(z[:], za[:], 0.0)
                    tmp4 = zp.tile([128, NCLS, HID], F32, tag="tmp4")
                    nc.vector.tensor_tensor(
                        out=tmp4[:],
                        in0=w2rep[:].rearrange("p (c i) -> p c i", i=HID),
                        in1=z[:].unsqueeze(1).broadcast_to([128, NCLS, HID]),
                        op=mybir.AluOpType.mult)
                    lg = zp.tile([128, NCLS], F32, tag="lg")
                    nc.vector.tensor_reduce(lg[:], tmp4[:], axis=mybir.AxisListType.X,
                                            op=mybir.AluOpType.add)
                    nc.vector.tensor_add(lgc[:, j * NCLS:(j + 1) * NCLS], lg[:],
                                         b2rep4[:])
                nc.sync.dma_start(
                    lg_out[ch * GCH:(ch + 1) * GCH, :]
                    .rearrange("(j p) c -> p j c", p=128),
                    lgc[:].rearrange("p (j c) -> p j c", c=NCLS))
    nc.compile()
    return nc


# ----------------------------------------------------------------- host entry

_STATE = {}


def _get_runner():
    if "run" in _STATE:
        return _STATE["run"]
    import jax
    from concourse import bass2jax
    from jax.sharding import Mesh, PartitionSpec, NamedSharding
    from jax.experimental.shard_map import shard_map

    nc = build_nc()
    bass2jax.install_neuronx_cc_hook()
    partition_name = nc.partition_id_tensor.name if nc.partition_id_tensor else None
    in_names, out_names, out_avals = [], [], []
    for alloc in nc.m.functions[0].allocations:
        if not isinstance(alloc, mybir.MemoryLocationSet):
            continue
        name = alloc.memorylocations[0].name
        if alloc.kind == "ExternalInput":
            if name != partition_name:
                in_names.append(name)
        elif alloc.kind == "ExternalOutput":
            out_names.append(name)
            out_avals.append(jax.core.ShapedArray(
                tuple(alloc.tensor_shape), mybir.dt.np(alloc.dtype)))
    n_params = len(in_names)
    in_names_all = list(in_names) + out_names
    if partition_name is not None:
        in_names_all.append(partition_name)

    def _body(*args):
        operands = list(args)
        if partition_name is not None:
            operands.append(bass2jax.partition_id_tensor())
        return tuple(bass2jax._bass_exec_p.bind(
            *operands, out_avals=tuple(out_avals), in_names=tuple(in_names_all),
            out_names=tuple(out_names), lowering_input_output_aliases=(),
            sim_require_finite=True, sim_require_nnan=True, nc=nc))

    devices = jax.devices()[:R]
    mesh = Mesh(np.asarray(devices), ("core",))
    nin = n_params + len(out_names)
    fn = jax.jit(shard_map(_body, mesh=mesh,
                           in_specs=(PartitionSpec("core"),) * nin,
                           out_specs=(PartitionSpec("core"),) * len(out_names),
                           check_rep=False))
    sh = NamedSharding(mesh, PartitionSpec("core"))

    def run(in_maps):
        import jax as _jax
        cat = [np.concatenate([np.asarray(in_maps[c][nm]) for c in range(R)], axis=0)
               for nm in in_names]
        zeros = [np.zeros((R * av.shape[0], *av.shape[1:]), av.dtype)
                 for av in out_avals]
        args = [_jax.device_put(a, sh) for a in cat + zeros]
        res = fn(*args)
        _jax.block_until_ready(res)
        return {nm: np.asarray(res[i]).reshape(R, *out_avals[i].shape)
                for i, nm in enumerate(out_names)}, (fn, args)

    _STATE["run"] = run
    return run


def _make_in_maps(inputs):
    x = np.asarray(inputs["x"], np.float32)
    edge_index = np.asarray(inputs["edge_index"])
    edge_attr = np.asarray(inputs["edge_attr"], np.float32)
    candidates = np.asarray(inputs["candidates"])
    shared = _prep_shared({k: np.asarray(v, np.float32) for k, v in inputs.items()
                           if k.startswith(("c1_", "c2_", "m_"))})
    in_maps = []
    for r in range(R):
        m = dict(shared)
        m.update(_prep_core(r, x, edge_index, edge_attr, candidates))
        # rename shared keys to tensor names
        for ci in (1, 2):
            for a, b in (("w1", "w1"), ("w2", "w2"), ("b1c", "b1c"),
                         ("b2rep", "b2rep"), ("rootrep", "rootrep"),
                         ("biasrep", "biasrep")):
                m[f"{a}_{ci}"] = shared[f"{b}_{ci}"]
        in_maps.append(m)
    return in_maps


def kernel(**inputs):
    run = _get_runner()
    in_maps = _make_in_maps(inputs)
    out, _ = run(in_maps)
    candidates = np.asarray(inputs["candidates"])
    # assemble h [N, 32] from core 0's h_out (all cores identical post-AG)
    hrows = out["h_out"][0]
    nid = np.arange(N)
    h = hrows[_frow(nid)][:, :HID].astype(np.float32)
    lg = out["lg_out"]                        # [R, PPAD, 4]
    edge_logits = np.concatenate([lg[r][:P // R] for r in range(R)], axis=0)
    return edge_logits.astype(np.float32), candidates, h


# revision 8
# speedup vs baseline: 1.0563x; 1.0563x over previous
"""Trainium2 Bass kernel for ConstraintPredictorGNN (2x NNConv + candidate MLP).

Strategy: shard destination nodes (2500/core) + their incoming edges across 8
cores; edges bin-packed into 21 windows of <=128 dst nodes / <=1024 edges,
aggregated via one-hot matmuls in PSUM; root term via per-window pseudo-edge
tiles. AllGather of node features between convs. Candidate pairs sharded
8x65536 (padded), pairwise features gathered with dma_gather from a packed
[20480, 64] u-table. Edge-MLP matmuls run as float32r (full PE rate).
"""
import sys
sys.path.insert(0, '/opt/trn_rl_repo')
import numpy as np
from contextlib import ExitStack

import concourse.bass as bass
import concourse.tile as tile
from concourse import bacc, mybir
from concourse.library_config import mlp as mlp_lib

F32 = mybir.dt.float32
F32R = mybir.dt.float32r
I16 = mybir.dt.int16
I32 = mybir.dt.int32

N, E, P = 20000, 150000, 500000
IN_CH = HID = 32
EDGE_DIM = 16
NCLS = 4
R = 8                      # cores
OWN = N // R               # 2500 owned nodes / core
OWNP = 2560                # padded own rows (AllGather block)
NODEROWS = OWNP * R        # 20480 rows in gathered tables
NW = 21                    # windows per core
WCAP = 1024                # edge capacity per window (8 tiles)
WT = 9                     # tiles per window (8 edge + 1 pseudo)
TILES = NW * WT            # 189
EPAD = NW * WCAP           # 21504 edge slots
SLOTS = TILES * 128        # 24192 gather rows for conv2
PPAD = 65536               # padded candidate pairs per core (8 x 8192)
GCH = 8192                 # dma_gather chunk
OOB = 1 << 20


def _frow(n):
    """node id -> row in the rank-concatenated [NODEROWS, 64] tables."""
    return (n // OWN) * OWNP + (n % OWN)


def _pack_windows(deg):
    """FFD bin-pack own-local nodes into NW windows (<=128 nodes, <=WCAP edges).
    Returns list of node-id lists (own-local ids)."""
    order = np.argsort(-deg, kind="stable")
    be = np.zeros(NW, np.int64)
    bn = np.zeros(NW, np.int64)
    wnodes = [[] for _ in range(NW)]
    for nl in order:
        d = int(deg[nl])
        # best-fit: among bins with room, pick fewest nodes (ties: most edge room)
        cand = [w for w in range(NW) if be[w] + d <= WCAP and bn[w] < 128]
        if not cand:
            raise AssertionError("window packing failed")
        w = min(cand, key=lambda w: (bn[w], be[w]))
        be[w] += d
        bn[w] += 1
        wnodes[w].append(int(nl))
    return wnodes


def _prep_core(r, x, edge_index, edge_attr, candidates):
    src, dst = edge_index[0], edge_index[1]
    lo, hi = r * OWN, (r + 1) * OWN
    mask = (dst >= lo) & (dst < hi)
    esel = np.nonzero(mask)[0]
    dloc = dst[esel] - lo
    deg = np.bincount(dloc, minlength=OWN)
    wnodes = _pack_windows(deg)

    # per-node edge lists (own-local)
    order = np.argsort(dloc, kind="stable")
    esorted = esel[order]
    starts = np.searchsorted(dloc[order], np.arange(OWN))
    ends = np.searchsorted(dloc[order], np.arange(OWN) + 1)

    eaT = np.zeros((EDGE_DIM, EPAD), np.float32)
    xs1 = np.zeros((128, TILES * IN_CH), np.float32)
    dstloc = np.full((128, TILES), -1.0, np.float32)
    gidx_rows = np.zeros(SLOTS, np.int64)          # table rows for conv2 gather
    slotnode = np.full((128, NW), OOB, np.int32)   # scatter target (own-local)

    for w, nodes in enumerate(wnodes):
        eids = np.concatenate([esorted[starts[nl]:ends[nl]] for nl in nodes]) \
            if nodes else np.zeros(0, np.int64)
        slot_of = {nl: j for j, nl in enumerate(nodes)}
        ne = len(eids)
        assert ne <= WCAP
        ebase = w * WCAP
        eaT[:, ebase:ebase + ne] = edge_attr[eids].T
        dl = np.array([slot_of[int(d) - lo] for d in dst[eids]], np.float64)
        es = src[eids]
        for s in range(8):
            tl = w * WT + s
            seg = slice(s * 128, min((s + 1) * 128, ne))
            k = max(0, seg.stop - seg.start)
            if k > 0:
                dstloc[:k, tl] = dl[seg]
                xs1[:k, tl * IN_CH:(tl + 1) * IN_CH] = x[es[seg]]
                gidx_rows[tl * 128:tl * 128 + k] = _frow(es[seg] + 0)
        # pseudo tile
        tl = w * WT + 8
        nn = len(nodes)
        gl = np.array(nodes, np.int64) + lo
        dstloc[:nn, tl] = np.arange(nn)
        xs1[:nn, tl * IN_CH:(tl + 1) * IN_CH] = x[gl]
        gidx_rows[tl * 128:tl * 128 + nn] = _frow(gl)
        slotnode[:nn, w] = np.array(nodes, np.int32)

    # wrap gather indices: one dma_gather call per window (1152 idx each)
    def wrap(ix):
        n = len(ix)
        return np.tile(ix.astype(np.int16).reshape(n // 16, 16).T, (8, 1))
    g2 = np.concatenate(
        [wrap(gidx_rows[w * WT * 128:(w + 1) * WT * 128]) for w in range(NW)],
        axis=1)  # [128, 21*72]

    # candidates shard
    c = candidates[r * (P // R):(r + 1) * (P // R)]
    c0 = np.zeros(PPAD, np.int64)
    c1 = np.zeros(PPAD, np.int64)
    c0[:len(c)] = _frow(c[:, 0].astype(np.int64))
    c1[:len(c)] = _frow(c[:, 1].astype(np.int64))
    gc = np.concatenate(
        [wrap(c0[i * GCH:(i + 1) * GCH]) for i in range(PPAD // GCH)] +
        [wrap(c1[i * GCH:(i + 1) * GCH]) for i in range(PPAD // GCH)], axis=1)

    return dict(eaT=eaT, xs1=xs1, dstloc=dstloc, gidx2=g2, slotnode=slotnode,
                candidx=gc)


def _prep_shared(ws):
    def rep(v, n=128):
        return np.tile(np.asarray(v, np.float32).reshape(1, -1), (n, 1))
    out = {}
    for ci, pre in ((1, "c1"), (2, "c2")):
        w1 = ws[f"{pre}_w1"]; b1 = ws[f"{pre}_b1"]
        w2 = ws[f"{pre}_w2"]; b2 = ws[f"{pre}_b2"]
        root = ws[f"{pre}_root"]; bias = ws[f"{pre}_bias"]
        perm = (np.arange(1024).reshape(32, 32).T.reshape(-1))  # io -> oi order
        w2p = w2[:, perm]
        out[f"w1_{ci}"] = np.ascontiguousarray(w1)                       # [16,1024]
        out[f"w2_{ci}"] = np.ascontiguousarray(
            w2p.reshape(8, 128, 1024).transpose(1, 0, 2).reshape(128, 8192))
        out[f"b1c_{ci}"] = np.ascontiguousarray(b1.reshape(8, 128).T)    # [128,8]
        out[f"b2rep_{ci}"] = rep(b2[perm])                               # [128,1024]
        out[f"rootrep_{ci}"] = rep(root.T.reshape(-1))                   # [128,1024]
        out[f"biasrep_{ci}"] = rep(bias)                                 # [128,32]
    mw1 = np.concatenate([ws["m_w1"][:HID, :], ws["m_w1"][HID:, :]], axis=1)
    out["mw1"] = np.ascontiguousarray(mw1)                               # [32,64]
    out["b1rep64"] = rep(np.concatenate([ws["m_b1"], np.zeros(HID, np.float32)]))
    out["w2rep"] = rep(ws["m_w2"].T.reshape(-1))                         # [128,128]
    out["b2rep4"] = rep(ws["m_b2"])                                      # [128,4]
    out["iota"] = np.tile(np.arange(128, dtype=np.float32)[None, :], (128, 1))
    out["ident"] = np.eye(128, dtype=np.float32)
    return out


# ----------------------------------------------------------------- device code

def _conv(nc, tc, ctx, cidx, *, w1, w2r, b1c, b2rep, rootrep, biasrep,
          ea_dram, dstloc, iota_f, get_xs, h_slots, slotnode_sb, h_dense):
    """One NNConv layer: writes relu(agg + root + bias) rows into h_dense.
    get_xs(w) -> accessor(s) giving the [128, 32] source-feature AP of tile s."""
    pools = ExitStack()
    h1p = pools.enter_context(tc.tile_pool(name=f"h1p{cidx}", bufs=1))
    eap = pools.enter_context(tc.tile_pool(name=f"eap{cidx}", bufs=2))
    wp = pools.enter_context(tc.tile_pool(name=f"wp{cidx}", bufs=3))
    mp = pools.enter_context(tc.tile_pool(name=f"mp{cidx}", bufs=3))
    ps_h1 = pools.enter_context(tc.tile_pool(name=f"psh1_{cidx}", bufs=2, space="PSUM"))
    ps_w = pools.enter_context(tc.tile_pool(name=f"psw{cidx}", bufs=2, space="PSUM"))
    ps_a = pools.enter_context(tc.tile_pool(name=f"psa{cidx}", bufs=2, space="PSUM"))

    nc.vector.memset(h_slots[:], 0.0)
    for w in range(NW):
        agg = ps_a.tile([128, HID], F32, tag="agg")
        ea_w = eap.tile([EDGE_DIM, WCAP], F32R, tag="ea_w", name="ea_w")
        nc.sync.dma_start(ea_w[:], ea_dram[:, w * WCAP:(w + 1) * WCAP])
        xs_of = get_xs(w)
        h1t = [None, None]
        for st in range(2):
            # layer1: h1T chunks for 512 edges
            h1t[st] = h1p.tile([128, 8, 512], F32R, tag=f"h1t{st}", name=f"h1t{st}")
            for kc in range(8):
                p1 = ps_h1.tile([128, 512], F32, tag="p1")
                nc.tensor.matmul(
                    p1[:], w1[:, kc * 128:(kc + 1) * 128],
                    ea_w[:, st * 512:(st + 1) * 512],
                    start=True, stop=True)
                nc.scalar.activation(h1t[st][:, kc, :], p1[:],
                                     mybir.ActivationFunctionType.Relu,
                                     bias=b1c[:, kc:kc + 1])
        for s in range(WT):
            t = w * WT + s
            if s < 8:
                st, q = s // 4, s % 4
                # layer2: W = h1 @ w2 + b2 for this 128-edge tile
                pw = ps_w.tile([128, 1024], F32, tag="pw")
                for kc in range(8):
                    lhsT = h1t[st][:, kc, q * 128:(q + 1) * 128]
                    for hf in range(2):
                        nc.tensor.matmul(
                            pw[:, hf * 512:(hf + 1) * 512], lhsT,
                            w2r[:, kc * 1024 + hf * 512: kc * 1024 + (hf + 1) * 512],
                            start=(kc == 0), stop=(kc == 7))
                wt = wp.tile([128, 1024], F32, tag="wt")
                nc.vector.tensor_add(wt[:], pw[:], b2rep[:])
                wsrc = wt[:]
            else:
                wsrc = rootrep[:]
            xs = xs_of(s)
            tmp = mp.tile([128, HID, HID], F32, tag="tmp")
            nc.vector.tensor_tensor(
                out=tmp[:], in0=wsrc.rearrange("p (o i) -> p o i", i=HID),
                in1=xs.unsqueeze(1).broadcast_to([128, HID, HID]),
                op=mybir.AluOpType.mult)
            msg = mp.tile([128, HID], F32, tag="msg")
            nc.vector.tensor_reduce(msg[:], tmp[:], axis=mybir.AxisListType.X,
                                    op=mybir.AluOpType.add)
            oh = mp.tile([128, 128], F32, tag="oh")
            nc.vector.tensor_scalar(oh[:], iota_f[:], dstloc[:, t:t + 1], None,
                                    op0=mybir.AluOpType.is_equal)
            nc.tensor.matmul(agg[:], oh[:], msg[:], start=(s == 0), stop=(s == 8))
        hb = mp.tile([128, HID], F32, tag="hb")
        nc.vector.tensor_add(hb[:], agg[:], biasrep[:])
        nc.vector.tensor_scalar_max(h_slots[:, w * 64: w * 64 + HID], hb[:], 0.0)
        nc.gpsimd.indirect_dma_start(
            out=h_dense, in_=h_slots[:, w * 64:(w + 1) * 64],
            out_offset=bass.IndirectOffsetOnAxis(ap=slotnode_sb[:, w:w + 1], axis=0),
            in_offset=None, bounds_check=OWNP - 1, oob_is_err=False)
    pools.close()


def build_nc():
    nc = bacc.Bacc("TRN2", target_bir_lowering=False, debug=False, num_devices=R)
    di = {}

    def inp(name, shape, dt):
        di[name] = nc.dram_tensor(name, shape, dt, kind="ExternalInput").ap()
        return di[name]

    # per-core data
    inp("eaT", [EDGE_DIM, EPAD], F32R)
    inp("xs1", [128, TILES * IN_CH], F32)
    inp("dstloc", [128, TILES], F32)
    inp("gidx2", [128, SLOTS // 16], I16)
    inp("slotnode", [128, NW], I32)
    inp("candidx", [128, 2 * PPAD // 16], I16)
    # shared weights/consts
    for ci in (1, 2):
        inp(f"w1_{ci}", [EDGE_DIM, 1024], F32R)
        inp(f"w2_{ci}", [128, 8192], F32R)
        inp(f"b1c_{ci}", [128, 8], F32)
        inp(f"b2rep_{ci}", [128, 1024], F32)
        inp(f"rootrep_{ci}", [128, 1024], F32)
        inp(f"biasrep_{ci}", [128, HID], F32)
    inp("mw1", [HID, 2 * HID], F32R)
    inp("b1rep64", [128, 2 * HID], F32)
    inp("w2rep", [128, NCLS * HID], F32)
    inp("b2rep4", [128, NCLS], F32)
    inp("iota", [128, 128], F32)
    inp("ident", [128, 128], F32)

    h_out = nc.dram_tensor("h_out", [NODEROWS, 64], F32, kind="ExternalOutput").ap()
    lg_out = nc.dram_tensor("lg_out", [128, PPAD * NCLS // 128], F32,
                            kind="ExternalOutput").ap()

    h_dense1 = nc.dram_tensor("h_dense1", [OWNP, 64], F32).ap()
    h_dense2 = nc.dram_tensor("h_dense2", [OWNP, 64], F32).ap()
    h_full1 = nc.dram_tensor("h_full1", [NODEROWS, 64], F32, addr_space="Shared").ap()
    h_full2 = nc.dram_tensor("h_full2", [NODEROWS, 64], F32, addr_space="Shared").ap()
    u_full = nc.dram_tensor("u_full", [NODEROWS, 2 * HID], F32).ap()

    with tile.TileContext(nc) as tc, ExitStack() as top:
        cpool = top.enter_context(tc.tile_pool(name="consts", bufs=1))
        with tc.tile_critical():
            nc.gpsimd.load_library(mlp_lib)

        def load(name, dt=None, shape=None):
            t = cpool.tile(shape or [s for s in di[name].shape],
                           dt or di[name].dtype, tag=name, name=name)
            nc.sync.dma_start(t[:], di[name])
            return t

        iota_f = load("iota")
        ident = load("ident")
        slotnode_sb = load("slotnode")
        dstloc = load("dstloc")
        h_slots = cpool.tile([128, NW * 64], F32, tag="h_slots")

        # ---------------- conv1 ----------------
        with ExitStack() as c1s:
            bp = c1s.enter_context(tc.tile_pool(name="big1", bufs=1))
            def get_xs1(w):
                xw = bp.tile([128, WT * IN_CH], F32, tag="xs1w", name="xs1w", bufs=3)
                nc.sync.dma_start(
                    xw[:], di["xs1"][:, w * WT * IN_CH:(w + 1) * WT * IN_CH])
                return lambda s: xw[:, s * IN_CH:(s + 1) * IN_CH]
            w1 = load("w1_1")
            w2r = bp.tile([128, 8192], F32R, tag="w2r1")
            nc.sync.dma_start(w2r[:], di["w2_1"])
            b1c = load("b1c_1"); b2rep = load("b2rep_1")
            rootrep = load("rootrep_1"); biasrep = load("biasrep_1")
            _conv(nc, tc, c1s, 1, w1=w1[:], w2r=w2r[:], b1c=b1c[:], b2rep=b2rep,
                  rootrep=rootrep, biasrep=biasrep, ea_dram=di["eaT"],
                  dstloc=dstloc[:], iota_f=iota_f, get_xs=get_xs1,
                  h_slots=h_slots[:], slotnode_sb=slotnode_sb[:], h_dense=h_dense1)
            nc.gpsimd.collective_compute(
                "AllGather", mybir.AluOpType.bypass,
                replica_groups=[list(range(R))], ins=[h_dense1], outs=[h_full1])

            # ---------------- conv2 ----------------
            gidx2 = bp.tile([128, SLOTS // 16], I16, tag="gidx2", name="gidx2")
            nc.sync.dma_start(gidx2[:], di["gidx2"])
            NWI = WT * 128 // 16   # idx cols per window (72)
            def get_xs2(w):
                xw = bp.tile([128, WT, 64], F32, tag="xs2w", name="xs2w", bufs=3)
                nc.gpsimd.dma_gather(
                    out_ap=xw[:], in_ap=h_full1,
                    idxs_ap=gidx2[:, w * NWI:(w + 1) * NWI],
                    num_idxs=WT * 128, num_idxs_reg=WT * 128, elem_size=64,
                    single_packet=False)
                return lambda s: xw[:, s, 0:HID]
            w1b = load("w1_2")
            w2rb = bp.tile([128, 8192], F32R, tag="w2r1")  # reuse slot
            nc.sync.dma_start(w2rb[:], di["w2_2"])
            b1cb = load("b1c_2"); b2repb = load("b2rep_2")
            rootrepb = load("rootrep_2"); biasrepb = load("biasrep_2")
            _conv(nc, tc, c1s, 2, w1=w1b[:], w2r=w2rb[:], b1c=b1cb[:], b2rep=b2repb,
                  rootrep=rootrepb, biasrep=biasrepb, ea_dram=di["eaT"],
                  dstloc=dstloc[:], iota_f=iota_f, get_xs=get_xs2,
                  h_slots=h_slots[:], slotnode_sb=slotnode_sb[:], h_dense=h_dense2)
            nc.gpsimd.collective_compute(
                "AllGather", mybir.AluOpType.bypass,
                replica_groups=[list(range(R))], ins=[h_dense2], outs=[h_full2])
            nc.sync.dma_start(h_out, h_full2)

        # ---------------- u = [h @ mw1_top + b1, h @ mw1_bot] ----------------
        with ExitStack() as us:
            up = us.enter_context(tc.tile_pool(name="upool", bufs=3))
            ups = us.enter_context(tc.tile_pool(name="upsum", bufs=2, space="PSUM"))
            mw1 = load("mw1")
            b1rep64 = load("b1rep64")
            for ti in range(NODEROWS // 128):
                ht = up.tile([128, 64], F32, tag="ht")
                nc.sync.dma_start(ht[:], h_full2[ti * 128:(ti + 1) * 128, :])
                pt = ups.tile([32, 128], F32, tag="pt")
                nc.tensor.transpose(pt[:], ht[:, 0:HID], ident[:])
                hT = up.tile([32, 128], F32R, tag="hT")
                nc.vector.tensor_copy(hT[:], pt[:])
                pu = ups.tile([128, 64], F32, tag="pu")
                nc.tensor.matmul(pu[:], hT[:], mw1[:], start=True, stop=True)
                ut = up.tile([128, 64], F32, tag="ut")
                nc.vector.tensor_add(ut[:], pu[:], b1rep64[:])
                nc.sync.dma_start(u_full[ti * 128:(ti + 1) * 128, :], ut[:])

        # ---------------- candidate MLP ----------------
        with ExitStack() as cs:
            gp = cs.enter_context(tc.tile_pool(name="gp", bufs=2))
            zp = cs.enter_context(tc.tile_pool(name="zp", bufs=2))
            cip = cs.enter_context(tc.tile_pool(name="cip", bufs=1))
            candidx = cip.tile([128, 2 * PPAD // 16], I16, tag="candidx", name="candidx")
            nc.sync.dma_start(candidx[:], di["candidx"])
            w2rep = load("w2rep")
            b2rep4 = load("b2rep4")
            NCHUNK = PPAD // GCH
            for ch in range(NCHUNK):
                g0 = gp.tile([128, GCH // 128, 64], F32, tag="g0")
                g1 = gp.tile([128, GCH // 128, 64], F32, tag="g1")
                for g, base in ((g0, 0), (g1, PPAD // 16)):
                    nc.gpsimd.dma_gather(
                        out_ap=g[:], in_ap=u_full,
                        idxs_ap=candidx[:, base + ch * (GCH // 16):
                                        base + (ch + 1) * (GCH // 16)],
                        num_idxs=GCH, num_idxs_reg=GCH, elem_size=64,
                        single_packet=False)
                J = GCH // 128
                za = zp.tile([128, J, HID], F32, tag="za")
                nc.vector.tensor_add(za[:], g0[:, :, 0:HID], g1[:, :, HID:64])
                z = zp.tile([128, J, HID], F32, tag="z")
                nc.scalar.activation(z[:], za[:],
                                     mybir.ActivationFunctionType.Relu)
                tmp4 = zp.tile([128, J, NCLS, HID], F32, tag="tmp4", bufs=1)
                nc.vector.tensor_tensor(
                    out=tmp4[:],
                    in0=w2rep[:].rearrange("p (c i) -> p c i", i=HID)
                        .unsqueeze(1).broadcast_to([128, J, NCLS, HID]),
                    in1=z[:].unsqueeze(2).broadcast_to([128, J, NCLS, HID]),
                    op=mybir.AluOpType.mult)
                lg = zp.tile([128, J, NCLS], F32, tag="lg")
                nc.vector.tensor_reduce(lg[:], tmp4[:], axis=mybir.AxisListType.X,
                                        op=mybir.AluOpType.add)
                lgc = zp.tile([128, J * NCLS], F32, tag="lgc")
                nc.vector.tensor_add(
                    lgc[:].rearrange("p (j c) -> p j c", c=NCLS), lg[:],
                    b2rep4[:].unsqueeze(1).broadcast_to([128, J, NCLS]))
                nc.sync.dma_start(
                    lg_out[:, ch * J * NCLS:(ch + 1) * J * NCLS], lgc[:])
    nc.compile()
    return nc


# ----------------------------------------------------------------- host entry

_STATE = {}


def _get_runner():
    if "run" in _STATE:
        return _STATE["run"]
    import jax
    from concourse import bass2jax
    from jax.sharding import Mesh, PartitionSpec, NamedSharding
    from jax.experimental.shard_map import shard_map

    nc = build_nc()
    bass2jax.install_neuronx_cc_hook()
    partition_name = nc.partition_id_tensor.name if nc.partition_id_tensor else None
    in_names, out_names, out_avals = [], [], []
    for alloc in nc.m.functions[0].allocations:
        if not isinstance(alloc, mybir.MemoryLocationSet):
            continue
        name = alloc.memorylocations[0].name
        if alloc.kind == "ExternalInput":
            if name != partition_name:
                in_names.append(name)
        elif alloc.kind == "ExternalOutput":
            out_names.append(name)
            out_avals.append(jax.core.ShapedArray(
                tuple(alloc.tensor_shape), mybir.dt.np(alloc.dtype)))
    n_params = len(in_names)
    in_names_all = list(in_names) + out_names
    if partition_name is not None:
        in_names_all.append(partition_name)

    def _body(*args):
        operands = list(args)
        if partition_name is not None:
            operands.append(bass2jax.partition_id_tensor())
        return tuple(bass2jax._bass_exec_p.bind(
            *operands, out_avals=tuple(out_avals), in_names=tuple(in_names_all),
            out_names=tuple(out_names), lowering_input_output_aliases=(),
            sim_require_finite=True, sim_require_nnan=True, nc=nc))

    devices = jax.devices()[:R]
    mesh = Mesh(np.asarray(devices), ("core",))
    nin = n_params + len(out_names)
    fn = jax.jit(shard_map(_body, mesh=mesh,
                           in_specs=(PartitionSpec("core"),) * nin,
                           out_specs=(PartitionSpec("core"),) * len(out_names),
                           check_rep=False))
    sh = NamedSharding(mesh, PartitionSpec("core"))

    def run(in_maps):
        import jax as _jax
        cat = [np.concatenate([np.asarray(in_maps[c][nm]) for c in range(R)], axis=0)
               for nm in in_names]
        zeros = [np.zeros((R * av.shape[0], *av.shape[1:]), av.dtype)
                 for av in out_avals]
        args = [_jax.device_put(a, sh) for a in cat + zeros]
        res = fn(*args)
        _jax.block_until_ready(res)
        return {nm: np.asarray(res[i]).reshape(R, *out_avals[i].shape)
                for i, nm in enumerate(out_names)}, (fn, args)

    _STATE["run"] = run
    return run


def _make_in_maps(inputs):
    x = np.asarray(inputs["x"], np.float32)
    edge_index = np.asarray(inputs["edge_index"])
    edge_attr = np.asarray(inputs["edge_attr"], np.float32)
    candidates = np.asarray(inputs["candidates"])
    shared = _prep_shared({k: np.asarray(v, np.float32) for k, v in inputs.items()
                           if k.startswith(("c1_", "c2_", "m_"))})
    in_maps = []
    for r in range(R):
        m = dict(shared)
        m.update(_prep_core(r, x, edge_index, edge_attr, candidates))
        # rename shared keys to tensor names
        for ci in (1, 2):
            for a, b in (("w1", "w1"), ("w2", "w2"), ("b1c", "b1c"),
                         ("b2rep", "b2rep"), ("rootrep", "rootrep"),
                         ("biasrep", "biasrep")):
                m[f"{a}_{ci}"] = shared[f"{b}_{ci}"]
        in_maps.append(m)
    return in_maps


def kernel(**inputs):
    run = _get_runner()
    in_maps = _make_in_maps(inputs)
    out, _ = run(in_maps)
    candidates = np.asarray(inputs["candidates"])
    # assemble h [N, 32] from core 0's h_out (all cores identical post-AG)
    hrows = out["h_out"][0]
    nid = np.arange(N)
    h = hrows[_frow(nid)][:, :HID].astype(np.float32)
    lg = out["lg_out"]                        # [R, 128, PPAD*4/128]
    NCH = PPAD // GCH
    J = GCH // 128
    shards = []
    for r in range(R):
        a = lg[r].reshape(128, NCH, J, NCLS).transpose(1, 2, 0, 3).reshape(PPAD, NCLS)
        shards.append(a[:P // R])
    edge_logits = np.concatenate(shards, axis=0)
    return edge_logits.astype(np.float32), candidates, h
